# revision 1
# baseline (speedup 1.0000x reference)
"""Causal GQA self-attention (RMS-normed QK + RoPE + softmax + proj) on 8 trn2 cores.

Sharding: core c = (batch b = c//2, head-group g = c%2).  Each core computes
batch b, q-heads {8g..8g+7}, kv-heads {2g, 2g+1}, and a partial output
projection using Wproj columns for those heads; the host sums the two
partials per batch.

Device layout notes (per core):
 - All matmul operands bf16; accumulation fp32 in PSUM.
 - x is pre-transposed on host: xT [D=1024, S=2048].
 - Q/K are projected into [s, dim] layout (for free-dim RMS-norm + RoPE),
   then PE-transposed into [dim, s] for attention.
 - Local q-dim order pairs head m of group0 with head m of group1:
   [h0,h4,h1,h5,h2,h6,h3,h7] so scores for the two kv-heads can be computed
   with two row-tiled (K=64) matmuls sharing the PE array.
 - Scores are computed TRANSPOSED: S^T [k_s, (pair m, q)] so exp output
   feeds PV directly as the moving operand; V is augmented with a ones
   column so row 64 of the PV accumulator is the softmax denominator.
 - rsqrt for RMS norm = exp(-0.5*ln(v)) so the only ACT tables used are
   exp/ln (one table set, no reload churn between softmax exps).
"""

import numpy as np
import ml_dtypes

B, S, D = 4, 2048, 1024
H, KVH, HD = 16, 4, 64
SC = S // 128   # 16 sequence chunks
DC = D // 128   # 8 d_model chunks
QD = 512        # local q dims (8 heads)
EPS = float(np.finfo(np.float32).eps)
ROPE_BASE = 10000.0

_NC_CACHE = {}
_LAST = None  # BassKernelResults of the last run (for test harness introspection)


def _build_bass():
    import concourse.bacc as bacc
    import concourse.mybir as mybir
    import concourse.tile as tile
    from concourse.masks import make_identity

    dt = mybir.dt
    f32, bf16 = dt.float32, dt.bfloat16
    Alu = mybir.AluOpType
    Act = mybir.ActivationFunctionType
    Ax = mybir.AxisListType

    nc = bacc.Bacc("TRN2", target_bir_lowering=False)

    xTd = nc.dram_tensor("xT", [D, S], bf16, kind="ExternalInput")
    wqd = nc.dram_tensor("wq", [D, QD], bf16, kind="ExternalInput")
    wkvd = nc.dram_tensor("wkv", [D, 256], bf16, kind="ExternalInput")
    wpd = nc.dram_tensor("wp", [QD, D], bf16, kind="ExternalInput")
    cqd = nc.dram_tensor("cq", [S, QD], bf16, kind="ExternalInput")
    sqd = nc.dram_tensor("sq", [S, QD], bf16, kind="ExternalInput")
    ckd = nc.dram_tensor("ck", [S, 128], bf16, kind="ExternalInput")
    skd = nc.dram_tensor("sk", [S, 128], bf16, kind="ExternalInput")
    yd = nc.dram_tensor("y", [S, D], f32, kind="ExternalOutput")

    with tile.TileContext(nc) as tc:
        with (
            tc.tile_pool(name="per", bufs=1) as per,
        ):
            xt = per.tile([128, DC * S], bf16, tag="xt")
            wq = per.tile([128, DC * QD], bf16, tag="wq")
            wkv = per.tile([128, DC * 256], bf16, tag="wkv")
            wp = per.tile([128, 4 * D], bf16, tag="wp")
            cq = per.tile([128, SC * QD], bf16, tag="cq")
            sq = per.tile([128, SC * QD], bf16, tag="sq")
            ck = per.tile([128, SC * 128], bf16, tag="ck")
            sk = per.tile([128, SC * 128], bf16, tag="sk")
            ident = per.tile([128, 128], bf16, tag="ident")
            eps_t = per.tile([128, 1], f32, tag="eps")
            half_t = per.tile([128, 1], f32, tag="half")
            QT = per.tile([128, 4 * S], bf16, tag="QT")
            KT = per.tile([128, S], bf16, tag="KT")
            VV = per.tile([128, SC * 130], bf16, tag="VV")
            YT = per.tile([128, 4 * S], bf16, tag="YT")

            make_identity(nc, ident[:])
            nc.vector.memset(eps_t[:], EPS)
            nc.vector.memset(half_t[:], 0.5)

            for k in range(DC):
                nc.sync.dma_start(xt[:, k * S:(k + 1) * S], xTd[k * 128:(k + 1) * 128, :])
                nc.sync.dma_start(wq[:, k * QD:(k + 1) * QD], wqd[k * 128:(k + 1) * 128, :])
                nc.sync.dma_start(wkv[:, k * 256:(k + 1) * 256], wkvd[k * 128:(k + 1) * 128, :])
            for m in range(4):
                nc.sync.dma_start(wp[:, m * D:(m + 1) * D], wpd[m * 128:(m + 1) * 128, :])
            for i in range(SC):
                nc.sync.dma_start(cq[:, i * QD:(i + 1) * QD], cqd[i * 128:(i + 1) * 128, :])
                nc.sync.dma_start(sq[:, i * QD:(i + 1) * QD], sqd[i * 128:(i + 1) * 128, :])
                nc.sync.dma_start(ck[:, i * 128:(i + 1) * 128], ckd[i * 128:(i + 1) * 128, :])
                nc.sync.dma_start(sk[:, i * 128:(i + 1) * 128], skd[i * 128:(i + 1) * 128, :])

            # ================= Phase 1: QKV projections + norm + rope =================
            with (
                tc.tile_pool(name="wk", bufs=3) as wk,
                tc.tile_pool(name="psq", bufs=3, space="PSUM") as psq,
                tc.tile_pool(name="pskv", bufs=3, space="PSUM") as pskv,
                tc.tile_pool(name="pst", bufs=2, space="PSUM") as pst,
            ):
              def prepA(i):
                q_ps = psq.tile([128, QD], f32, tag="q")
                for k in range(DC):
                    nc.tensor.matmul(
                        q_ps[:],
                        xt[:, k * S + i * 128: k * S + (i + 1) * 128],
                        wq[:, k * QD:(k + 1) * QD],
                        start=(k == 0), stop=(k == DC - 1),
                    )
                kv_ps = pskv.tile([128, 256], f32, tag="kv")
                for k in range(DC):
                    nc.tensor.matmul(
                        kv_ps[:],
                        xt[:, k * S + i * 128: k * S + (i + 1) * 128],
                        wkv[:, k * 256:(k + 1) * 256],
                        start=(k == 0), stop=(k == DC - 1),
                    )
                q2 = wk.tile([128, QD], f32, tag="q2")
                nc.scalar.activation(q2[:], q_ps[:], Act.Square)
                k2 = wk.tile([128, 128], f32, tag="k2")
                nc.scalar.activation(k2[:], kv_ps[:, 0:128], Act.Square)
                ss = wk.tile([128, 10], f32, tag="ss")
                nc.vector.tensor_reduce(
                    ss[:, 0:8], q2[:].rearrange("p (h f) -> p h f", h=8), Ax.X, Alu.add)
                nc.vector.tensor_reduce(
                    ss[:, 8:10], k2[:].rearrange("p (h f) -> p h f", h=2), Ax.X, Alu.add)
                lnv = wk.tile([128, 10], f32, tag="lnv")
                nc.vector.tensor_scalar(lnv[:], ss[:], 1.0 / HD, EPS, Alu.mult, Alu.add)
                # rsqrt(v): ACT-exp seed exp(0.5-0.5v) ~ v^-0.5 near 1, then
                # 3 Newton steps y <- y*(1.5 - 0.5*v*y^2) on DVE (v in ~[0.4, 2.2])
                rs = wk.tile([128, 10], f32, tag="rs")
                nc.scalar.activation(rs[:], lnv[:], Act.Exp, scale=-0.5, bias=half_t[:])
                nt = wk.tile([128, 30], f32, tag="nt")
                for it in range(3):
                    t0 = nt[:, it * 30 // 3: it * 30 // 3 + 10]
                    nc.vector.tensor_tensor(t0, rs[:], rs[:], Alu.mult)
                    nc.vector.tensor_tensor(t0, t0, lnv[:], Alu.mult)
                    nc.vector.tensor_scalar(t0, t0, -0.5, 1.5, Alu.mult, Alu.add)
                    nc.vector.tensor_tensor(rs[:], rs[:], t0, Alu.mult)
                qn = wk.tile([128, QD], bf16, tag="qn", bufs=4)
                for h in range(8):
                    nc.vector.tensor_scalar_mul(
                        qn[:, h * 64:(h + 1) * 64], q_ps[:, h * 64:(h + 1) * 64],
                        rs[:, h:h + 1])
                kn = wk.tile([128, 128], bf16, tag="kn", bufs=4)
                for h in range(2):
                    nc.vector.tensor_scalar_mul(
                        kn[:, h * 64:(h + 1) * 64], kv_ps[:, h * 64:(h + 1) * 64],
                        rs[:, 8 + h:9 + h])
                vt = VV[:, i * 130:(i + 1) * 130]
                nc.vector.tensor_copy(vt[:, 0:64], kv_ps[:, 128:192])
                nc.vector.tensor_copy(vt[:, 65:129], kv_ps[:, 192:256])
                nc.vector.memset(vt[:, 64:65], 1.0)
                nc.vector.memset(vt[:, 129:130], 1.0)
                return qn, kn

              def prepB(i, qn, kn):
                r1 = wk.tile([128, QD], bf16, tag="r1")
                nc.gpsimd.tensor_tensor(r1[:], qn[:], cq[:, i * QD:(i + 1) * QD], Alu.mult)
                r2 = wk.tile([128, QD], bf16, tag="r2")
                qn3 = qn[:].rearrange("p (h t f) -> p h t f", t=2, f=32)
                sq3 = sq[:, i * QD:(i + 1) * QD].rearrange("p (h t f) -> p h t f", t=2, f=32)
                r23 = r2[:].rearrange("p (h t f) -> p h t f", t=2, f=32)
                nc.gpsimd.tensor_tensor(r23[:, :, 0, :], qn3[:, :, 1, :], sq3[:, :, 0, :], Alu.mult)
                nc.gpsimd.tensor_tensor(r23[:, :, 1, :], qn3[:, :, 0, :], sq3[:, :, 1, :], Alu.mult)
                qr = wk.tile([128, QD], bf16, tag="qr", bufs=4)
                nc.gpsimd.tensor_tensor(qr[:], r1[:], r2[:], Alu.add)
                rk1 = wk.tile([128, 128], bf16, tag="rk1")
                nc.gpsimd.tensor_tensor(rk1[:], kn[:], ck[:, i * 128:(i + 1) * 128], Alu.mult)
                rk2 = wk.tile([128, 128], bf16, tag="rk2")
                kn3 = kn[:].rearrange("p (h t f) -> p h t f", t=2, f=32)
                sk3 = sk[:, i * 128:(i + 1) * 128].rearrange("p (h t f) -> p h t f", t=2, f=32)
                rk23 = rk2[:].rearrange("p (h t f) -> p h t f", t=2, f=32)
                nc.gpsimd.tensor_tensor(rk23[:, :, 0, :], kn3[:, :, 1, :], sk3[:, :, 0, :], Alu.mult)
                nc.gpsimd.tensor_tensor(rk23[:, :, 1, :], kn3[:, :, 0, :], sk3[:, :, 1, :], Alu.mult)
                kr = wk.tile([128, 128], bf16, tag="kr", bufs=4)
                nc.gpsimd.tensor_tensor(kr[:], rk1[:], rk2[:], Alu.add)
                return qr, kr

              def prepC(i, qr, kr):
                for m in range(4):
                    t_ps = pst.tile([128, 128], bf16, tag="t")
                    nc.tensor.transpose(t_ps[:], qr[:, m * 128:(m + 1) * 128], ident[:])
                    nc.vector.tensor_copy(QT[:, m * S + i * 128: m * S + (i + 1) * 128], t_ps[:])
                t_ps = pst.tile([128, 128], bf16, tag="t")
                nc.tensor.transpose(t_ps[:], kr[:], ident[:])
                nc.vector.tensor_copy(KT[:, i * 128:(i + 1) * 128], t_ps[:])

              stage = {}
              for ii in range(SC + 2):
                if ii < SC:
                    stage[ii] = prepA(ii)
                if 1 <= ii and ii - 1 < SC:
                    stage[ii - 1] = prepB(ii - 1, *stage[ii - 1])
                if 2 <= ii:
                    prepC(ii - 2, *stage.pop(ii - 2))
            # ================= Phase 2: attention + output projection =================
            with (
                tc.tile_pool(name="wk2", bufs=3) as wk2,
                tc.tile_pool(name="ep", bufs=4) as ep,
                tc.tile_pool(name="pss", bufs=2, space="PSUM") as pss,
                tc.tile_pool(name="pso", bufs=2, space="PSUM") as pso,
                tc.tile_pool(name="psp", bufs=2, space="PSUM") as psp,
            ):
              for i in range(SC):
                # ---- attention over k-chunks j<=i ----
                oa = pso.tile([65, QD], f32, tag="o")
                ob = pso.tile([65, QD], f32, tag="o")
                qt0 = QT[0:64, :].rearrange("p (m s) -> p m s", m=4)[:, :, i * 128:(i + 1) * 128]
                qt1 = QT[64:128, :].rearrange("p (m s) -> p m s", m=4)[:, :, i * 128:(i + 1) * 128]
                for j in range(i + 1):
                    s_ps = pss.tile([128, 1024], f32, tag="s")
                    nc.tensor.matmul(s_ps[:, 0:512], KT[0:64, j * 128:(j + 1) * 128], qt0,
                                     start=True, stop=True)
                    nc.tensor.matmul(s_ps[:, 512:1024], KT[64:128, j * 128:(j + 1) * 128], qt1,
                                     start=True, stop=True)
                    et = ep.tile([128, 1024], bf16, tag="e")
                    nc.scalar.activation(et[:], s_ps[:], Act.Exp)
                    if j == i:
                        # zero strictly-above-diagonal scores (k > q) in-block
                        et3 = et[:].rearrange("p (b q) -> p b q", q=128)
                        nc.gpsimd.affine_select(
                            et3, et3, pattern=[[0, 8], [1, 128]],
                            compare_op=Alu.is_ge, fill=0.0, base=0,
                            channel_multiplier=-1)
                    nc.tensor.matmul(oa[:], VV[:, j * 130: j * 130 + 65], et[:, 0:512],
                                     start=(j == 0), stop=(j == i))
                    nc.tensor.matmul(ob[:], VV[:, j * 130 + 65: j * 130 + 130], et[:, 512:1024],
                                     start=(j == 0), stop=(j == i))

                # ---- normalize and write y^T ----
                rcs = []
                for g, o_ps in ((0, oa), (1, ob)):
                    rc = wk2.tile([1, QD], f32, tag="rc")
                    nc.vector.reciprocal(rc[:], o_ps[64:65, :])
                    rb = wk2.tile([64, QD], f32, tag="rb")
                    nc.gpsimd.partition_broadcast(rb[:], rc[:], channels=64)
                    rcs.append(rb)
                for g, o_ps in ((0, oa), (1, ob)):
                    out_ap = YT[g * 64:(g + 1) * 64, :].rearrange(
                        "p (m s) -> p m s", m=4)[:, :, i * 128:(i + 1) * 128]
                    nc.vector.tensor_tensor(
                        out_ap,
                        o_ps[0:64, :].rearrange("p (m q) -> p m q", m=4),
                        rcs[g][:].rearrange("p (m q) -> p m q", m=4),
                        Alu.mult)

                # ---- output projection, deferred one chunk for pipeline slack ----
                for ip in ([i - 1] if i < SC - 1 else [i - 1, i]):
                    if ip < 0:
                        continue
                    for dh in range(2):
                        op_ps = psp.tile([128, 512], f32, tag="op")
                        for m in range(4):
                            nc.tensor.matmul(
                                op_ps[:],
                                YT[:, m * S + ip * 128: m * S + (ip + 1) * 128],
                                wp[:, m * D + dh * 512: m * D + (dh + 1) * 512],
                                start=(m == 0), stop=(m == 3))
                        osb = wk2.tile([128, 512], f32, tag="osb")
                        nc.vector.tensor_copy(osb[:], op_ps[:])
                        nc.sync.dma_start(yd[ip * 128:(ip + 1) * 128, dh * 512:(dh + 1) * 512], osb[:])

    nc.compile()
    return nc


def _get_nc():
    if "nc" not in _NC_CACHE:
        _NC_CACHE["nc"] = _build_bass()
    return _NC_CACHE["nc"]


def _core_inputs(xb, Wq, Wk, Wv, Wproj, q_gain, g):
    bf = ml_dtypes.bfloat16
    qorder = [8 * g + o for o in (0, 4, 1, 5, 2, 6, 3, 7)]

    xT = np.ascontiguousarray(np.asarray(xb, np.float32).T).astype(bf)
    Wq_l = np.concatenate([Wq[h * 64:(h + 1) * 64] for h in qorder], 0)  # [512, D]
    wq = np.ascontiguousarray(Wq_l.T).astype(bf)
    Wk_l = Wk[2 * g * 64:(2 * g + 2) * 64]  # [128, D]
    Wv_l = Wv[2 * g * 64:(2 * g + 2) * 64]
    wkv = np.ascontiguousarray(np.concatenate([Wk_l, Wv_l], 0).T).astype(bf)
    cols = np.array([(8 * g + m + 4 * half) * 64 + f
                     for m in range(4) for half in range(2) for f in range(64)])
    wp = np.ascontiguousarray(Wproj[:, cols].T).astype(bf)  # [512, D]

    inv = (1.0 / (ROPE_BASE ** (np.arange(0, HD, 2, dtype=np.float32) / HD))).astype(np.float32)
    th = np.arange(S, dtype=np.float32)[:, None] * inv[None, :]
    cos, sin = np.cos(th).astype(np.float32), np.sin(th).astype(np.float32)
    cfull = np.concatenate([cos, cos], 1)       # [S, 64]
    sfull = np.concatenate([sin, -sin], 1)      # [S, 64] (signs baked)
    scale_q = np.asarray(q_gain, np.float32)[qorder] / np.float32(np.sqrt(HD))
    cq = np.concatenate([cfull * sc for sc in scale_q], 1).astype(bf)
    sq = np.concatenate([sfull * sc for sc in scale_q], 1).astype(bf)
    ck = np.concatenate([cfull, cfull], 1).astype(bf)
    sk = np.concatenate([sfull, sfull], 1).astype(bf)

    return {"xT": xT, "wq": wq, "wkv": wkv, "wp": wp,
            "cq": cq, "sq": sq, "ck": ck, "sk": sk}


def kernel(x, Wq, Wk, Wv, Wproj, q_gain):
    global _LAST
    x = np.asarray(x, np.float32)
    Wq = np.asarray(Wq, np.float32)
    Wk = np.asarray(Wk, np.float32)
    Wv = np.asarray(Wv, np.float32)
    Wproj = np.asarray(Wproj, np.float32)
    q_gain = np.asarray(q_gain, np.float32)

    nc = _get_nc()
    in_maps = []
    for c in range(8):
        b, g = divmod(c, 2)
        in_maps.append(_core_inputs(x[b], Wq, Wk, Wv, Wproj, q_gain, g))

    from concourse.bass_utils import run_bass_kernel_spmd
    res = run_bass_kernel_spmd(nc, in_maps, core_ids=list(range(8)))
    _LAST = res

    y = np.empty((B, S, D), np.float32)
    for b in range(B):
        y[b] = res.results[2 * b]["y"] + res.results[2 * b + 1]["y"]
    return y

